# revision 4
# baseline (speedup 1.0000x reference)
"""Causal multi-head attention (dense transformer block) on 8 Trainium2 cores.

Problem: x[4, 2048, 1024], 16 heads, head_dim 64, causal softmax attention
with QKV + output projections (torch Linear layout weights).

Sharding: 8 cores = 4 batches x 2 head-groups (8 heads each).  Each core
computes QKV projection for its 8 heads, attention, and its partial output
projection (row-parallel over w_out).  Host sums the two partials per batch
and adds b_out.

All device layouts are "transposed" so no on-device transposes are needed:
  - x is fed as xT [d, s]; Q^T/K^T are produced as [head_dim, s]
  - scores are computed as S^T [k, q]; softmax runs along partitions via
    ones-matmul column sums; O is accumulated directly as O^T [e_loc, q],
    which is exactly the lhsT the output projection needs.
Matmul inputs are bf16 (PSUM accumulation is fp32); everything else fp32.
"""

import sys

sys.path.insert(0, "/opt/trn_rl_repo")

import numpy as np
import ml_dtypes

import concourse.bass as bass
import concourse.mybir as mybir
import concourse.tile as tile
from concourse import bacc
from concourse import bass_utils
from concourse.masks import make_upper_triangular

F32 = mybir.dt.float32
BF16 = mybir.dt.bfloat16
EXP = mybir.ActivationFunctionType.Exp

B, S, D = 4, 2048, 1024
HTOT, HD = 16, 64
NCORES = 8
HLOC = HTOT // 2          # heads per core
ELOC = HLOC * HD          # 512 local embedding width
NHP = HLOC // 2           # 4 head pairs
QC = 512                  # q-chunk width
NQC = S // QC             # 4
NKT = S // 128            # 16 k tiles over sequence
NDT = D // 128            # 8 k tiles over model dim
SCALE = 1.0 / float(np.sqrt(HD))

_CACHE = {}


def _build_nc():
    nc = bacc.Bacc("TRN2", target_bir_lowering=False, debug=False)

    xT = nc.dram_tensor("xT", [D, S], BF16, kind="ExternalInput")
    wqT = nc.dram_tensor("wqT", [D, ELOC], BF16, kind="ExternalInput")
    wkT = nc.dram_tensor("wkT", [D, ELOC], BF16, kind="ExternalInput")
    wvT = nc.dram_tensor("wvT", [D, ELOC], BF16, kind="ExternalInput")
    woT = nc.dram_tensor("woT", [ELOC, D], BF16, kind="ExternalInput")
    bqk = nc.dram_tensor("bqk", [128, 2, NHP], F32, kind="ExternalInput")
    bvb = nc.dram_tensor("bvb", [128, ELOC], F32, kind="ExternalInput")
    outp = nc.dram_tensor("outp", [S, D], F32, kind="ExternalOutput")

    with tile.TileContext(nc) as tc:
        with tc.tile_pool(name="const", bufs=1) as constp, \
             tc.tile_pool(name="wpool", bufs=1) as wp, \
             tc.tile_pool(name="qkv", bufs=1) as qkvp, \
             tc.tile_pool(name="xt", bufs=1) as xtp, \
             tc.tile_pool(name="pt", bufs=4) as ptp, \
             tc.tile_pool(name="otn", bufs=8) as otnp, \
             tc.tile_pool(name="dr", bufs=2) as drp, \
             tc.tile_pool(name="osb", bufs=4) as osbp:

            # ---- constants ----
            trimask = constp.tile([128, 128], BF16, name="trimask")
            make_upper_triangular(nc, trimask[:], val=1.0, diag=True)
            ones_a = constp.tile([128, 1], BF16, name="ones_a")
            nc.gpsimd.memset(ones_a[:], 1.0)
            ones_b = constp.tile([128, 2], BF16, name="ones_b")
            nc.gpsimd.memset(ones_b[:], 0.0)
            nc.gpsimd.memset(ones_b[:, 0:1], 1.0)
            bc_lhs = constp.tile([33, 128], BF16, name="bc_lhs")
            nc.gpsimd.memset(bc_lhs[:], 0.0)
            nc.gpsimd.memset(bc_lhs[0:1, 0:64], 1.0)
            nc.gpsimd.memset(bc_lhs[32:33, 64:128], 1.0)

            bqk_sb = constp.tile([128, 2, NHP], F32, name="bqk_sb")
            nc.sync.dma_start(bqk_sb[:], bqk[:])
            bvb_sb = constp.tile([128, ELOC], F32, name="bvb_sb")
            nc.sync.dma_start(bvb_sb[:], bvb[:])

            # ---- weights ----
            wq_sb, wk_sb, wv_sb = [], [], []
            for kt in range(NDT):
                for lst, src, nm in ((wq_sb, wqT, "wq"), (wk_sb, wkT, "wk"),
                                     (wv_sb, wvT, "wv")):
                    t = wp.tile([128, ELOC], BF16, name=f"{nm}{kt}")
                    nc.sync.dma_start(t[:], src[128 * kt:128 * (kt + 1), :])
                    lst.append(t)
            wo_sb = []
            for hp in range(NHP):
                t = wp.tile([128, D], BF16, name=f"wo{hp}")
                nc.sync.dma_start(t[:], woT[128 * hp:128 * (hp + 1), :])
                wo_sb.append(t)

            # ---- load xT ----
            xts = []
            for kt in range(NDT):
                t = xtp.tile([128, S], BF16, name=f"xt{kt}")
                nc.sync.dma_start(t[:], xT[128 * kt:128 * (kt + 1), :])
                xts.append(t)

            # ---- QKV projection ----
            QT, KT = [], []
            for hp in range(NHP):
                QT.append(qkvp.tile([128, S], BF16, name=f"qt{hp}"))
                KT.append(qkvp.tile([128, S], BF16, name=f"kt{hp}"))
            V = [qkvp.tile([128, ELOC], BF16, name=f"v{st}") for st in range(NKT)]

            with tc.tile_pool(name="psq", bufs=6, space="PSUM") as psq:
                for dst, wsb, col in ((QT, wq_sb, 0), (KT, wk_sb, 1)):
                    for hp in range(NHP):
                        for c in range(NQC):
                            ps = psq.tile([128, QC], F32)
                            for kt in range(NDT):
                                nc.tensor.matmul(
                                    ps[:],
                                    lhsT=wsb[kt][:, 128 * hp:128 * (hp + 1)],
                                    rhs=xts[kt][:, QC * c:QC * (c + 1)],
                                    start=(kt == 0), stop=(kt == NDT - 1))
                            nc.vector.tensor_scalar_add(
                                dst[hp][:, QC * c:QC * (c + 1)], ps[:],
                                bqk_sb[:, col, hp:hp + 1])
                for st in range(NKT):
                    ps = psq.tile([128, ELOC], F32)
                    for kt in range(NDT):
                        nc.tensor.matmul(
                            ps[:],
                            lhsT=xts[kt][:, 128 * st:128 * (st + 1)],
                            rhs=wv_sb[kt][:],
                            start=(kt == 0), stop=(kt == NDT - 1))
                    nc.vector.tensor_add(V[st][:], ps[:], bvb_sb[:])

            # ---- attention + output projection ----
            tri3 = trimask[:][:, None, :].broadcast_to([128, 2, 128])
            with tc.tile_pool(name="pss", bufs=2, space="PSUM") as pss, \
                 tc.tile_pool(name="pso", bufs=2, space="PSUM") as pso, \
                 tc.tile_pool(name="psd", bufs=1, space="PSUM") as psd, \
                 tc.tile_pool(name="psop", bufs=1, space="PSUM") as psop:
                for j in range(NQC):
                    otn_j = []
                    for hp in range(NHP):
                        ps_ot = pso.tile([128, QC], F32)
                        ps_d = psd.tile([128, QC], F32)
                        nkt = 4 * j + 4
                        for i in range(nkt):
                            w = 128 * (i - 4 * j) if i >= 4 * j else 0
                            last = (i == nkt - 1)
                            ps_s = pss.tile([128, 2, QC], F32)
                            for h2 in range(2):
                                nc.tensor.matmul(
                                    ps_s[:, h2, w:QC],
                                    lhsT=KT[hp][64 * h2:64 * (h2 + 1),
                                                128 * i:128 * (i + 1)],
                                    rhs=QT[hp][64 * h2:64 * (h2 + 1),
                                               QC * j + w:QC * (j + 1)],
                                    start=True, stop=True)
                            pt = ptp.tile([128, 2, QC], BF16)
                            nc.scalar.activation(pt[:, :, w:QC], ps_s[:, :, w:QC],
                                                 EXP, scale=SCALE)
                            if i >= 4 * j:
                                nc.vector.tensor_mul(
                                    pt[:, :, w:w + 128], pt[:, :, w:w + 128],
                                    tri3[:, :, :])
                            for h2 in range(2):
                                nc.tensor.matmul(
                                    ps_ot[64 * h2:64 * (h2 + 1), w:QC],
                                    lhsT=V[i][:, 64 * (2 * hp + h2):
                                              64 * (2 * hp + h2 + 1)],
                                    rhs=pt[:, h2, w:QC],
                                    start=(i == 0), stop=last,
                                    tile_position=(0, 64 * h2))
                            nc.tensor.matmul(ps_d[0:1, w:QC], lhsT=ones_a[:],
                                             rhs=pt[:, 0, w:QC],
                                             start=(i == 0), stop=last)
                            nc.tensor.matmul(ps_d[32:34, w:QC], lhsT=ones_b[:],
                                             rhs=pt[:, 1, w:QC],
                                             start=(i == 0), stop=last,
                                             tile_position=(0, 32))
                        # normalization: dr = 1/d ; broadcast via pattern matmul
                        dr = drp.tile([33, QC], BF16)
                        nc.gpsimd.memset(dr[:], 0.0)
                        with nc.allow_low_precision(reason="softmax denom bf16"):
                            nc.vector.reciprocal(dr[0:1, :], ps_d[0:1, :])
                            nc.vector.reciprocal(dr[32:33, :], ps_d[32:33, :])
                        nc.tensor.matmul(ps_d[:], lhsT=bc_lhs[:], rhs=dr[:],
                                         start=True, stop=True)
                        dbc = drp.tile([128, QC], BF16, name="dbc")
                        nc.vector.tensor_copy(dbc[:], ps_d[:])
                        otn = otnp.tile([128, QC], BF16)
                        nc.vector.tensor_mul(otn[:], ps_ot[:], dbc[:])
                        otn_j.append(otn)
                    # output projection for this q chunk
                    for m in range(4):
                        s0 = QC * j + 128 * m
                        for eo in range(2):
                            ps_o = psop.tile([128, 512], F32)
                            for hp in range(NHP):
                                nc.tensor.matmul(
                                    ps_o[:],
                                    lhsT=otn_j[hp][:, 128 * m:128 * (m + 1)],
                                    rhs=wo_sb[hp][:, 512 * eo:512 * (eo + 1)],
                                    start=(hp == 0), stop=(hp == NHP - 1))
                            osb = osbp.tile([128, 512], F32)
                            nc.vector.tensor_copy(osb[:], ps_o[:])
                            nc.sync.dma_start(
                                outp[s0:s0 + 128, 512 * eo:512 * (eo + 1)],
                                osb[:])
    nc.compile()
    return nc


def _get_nc():
    if "nc" not in _CACHE:
        _CACHE["nc"] = _build_nc()
    return _CACHE["nc"]


def _prep_core_inputs(x, w_qkv, b_qkv, w_out, b, hg):
    r0 = ELOC * hg
    wq = w_qkv[r0:r0 + ELOC, :]
    wk = w_qkv[D + r0:D + r0 + ELOC, :]
    wv = w_qkv[2 * D + r0:2 * D + r0 + ELOC, :]
    bq = b_qkv[r0:r0 + ELOC]
    bk = b_qkv[D + r0:D + r0 + ELOC]
    bv = b_qkv[2 * D + r0:2 * D + r0 + ELOC]

    bf = ml_dtypes.bfloat16
    bqk_arr = np.empty((128, 2, NHP), np.float32)
    bqk_arr[:, 0, :] = bq.reshape(NHP, 128).T
    bqk_arr[:, 1, :] = bk.reshape(NHP, 128).T
    return {
        "xT": np.ascontiguousarray(x[b].T).astype(bf),
        "wqT": np.ascontiguousarray(wq.T).astype(bf),
        "wkT": np.ascontiguousarray(wk.T).astype(bf),
        "wvT": np.ascontiguousarray(wv.T).astype(bf),
        "woT": np.ascontiguousarray(w_out[:, r0:r0 + ELOC].T).astype(bf),
        "bqk": bqk_arr,
        "bvb": np.tile(bv.astype(np.float32)[None, :], (128, 1)),
    }


def kernel(x, w_qkv, b_qkv, w_out, b_out, _trace=False, _trace_kwargs=None):
    x = np.asarray(x, np.float32)
    w_qkv = np.asarray(w_qkv, np.float32)
    b_qkv = np.asarray(b_qkv, np.float32)
    w_out = np.asarray(w_out, np.float32)
    b_out = np.asarray(b_out, np.float32)

    nc = _get_nc()
    in_maps = []
    for core in range(NCORES):
        b, hg = core // 2, core % 2
        in_maps.append(_prep_core_inputs(x, w_qkv, b_qkv, w_out, b, hg))

    kw = {}
    if _trace:
        kw.update(trace=True, **(_trace_kwargs or {}))
    import time
    res = None
    for attempt in range(4):
        try:
            res = bass_utils.run_bass_kernel_spmd(
                nc, in_maps, core_ids=list(range(NCORES)), **kw)
            break
        except Exception:
            if attempt == 3:
                raise
            # Transient axon/NRT device flake: reset the PJRT backend so the
            # retry starts from a clean client, like a fresh process would.
            try:
                import jax
                jax.clear_caches()
                import jax._src.xla_bridge as _xb
                _xb._clear_backends()
            except Exception:
                pass
            time.sleep(5.0 * (attempt + 1))

    out = np.empty((B, S, D), np.float32)
    for b in range(B):
        out[b] = res.results[2 * b]["outp"] + res.results[2 * b + 1]["outp"] \
            + b_out[None, :]
    if _trace:
        return out, res
    return out


# revision 9
# speedup vs baseline: 1.2084x; 1.2084x over previous
"""Causal multi-head attention (dense transformer block) on 8 Trainium2 cores.

Problem: x[4, 2048, 1024], 16 heads, head_dim 64, causal softmax attention
with QKV + output projections (torch Linear layout weights).

Sharding: 8 cores = 4 batches x 2 head-groups (8 heads each).  Each core
computes QKV projection for its 8 heads, attention, and its partial output
projection (row-parallel over w_out).  Host sums the two partials per batch
and adds b_out.

All device layouts are "transposed" so no on-device transposes are needed:
  - x is fed as xT [d, s]; Q^T/K^T are produced as [head_dim, s]
  - scores are computed as S^T [k, q]; softmax runs along partitions via
    ones-matmul column sums; O is accumulated directly as O^T [e_loc, q],
    which is exactly the lhsT the output projection needs.
Matmul inputs are bf16 (PSUM accumulation is fp32); everything else fp32.
"""

import sys

sys.path.insert(0, "/opt/trn_rl_repo")

import numpy as np
import ml_dtypes

import concourse.bass as bass
import concourse.mybir as mybir
import concourse.tile as tile
from concourse import bacc
from concourse import bass_utils
from concourse.masks import make_upper_triangular

F32 = mybir.dt.float32
BF16 = mybir.dt.bfloat16
EXP = mybir.ActivationFunctionType.Exp

B, S, D = 4, 2048, 1024
HTOT, HD = 16, 64
NCORES = 8
HLOC = HTOT // 2          # heads per core
ELOC = HLOC * HD          # 512 local embedding width
NHP = HLOC // 2           # 4 head pairs
QC = 512                  # q-chunk width
NQC = S // QC             # 4
NKT = S // 128            # 16 k tiles over sequence
NDT = D // 128            # 8 k tiles over model dim
SCALE = 1.0 / float(np.sqrt(HD))

_CACHE = {}


def _build_nc():
    nc = bacc.Bacc("TRN2", target_bir_lowering=False, debug=False)

    xT = nc.dram_tensor("xT", [D, S], BF16, kind="ExternalInput")
    wqT = nc.dram_tensor("wqT", [D, ELOC], BF16, kind="ExternalInput")
    wkT = nc.dram_tensor("wkT", [D, ELOC], BF16, kind="ExternalInput")
    wvT = nc.dram_tensor("wvT", [D, ELOC], BF16, kind="ExternalInput")
    woT = nc.dram_tensor("woT", [ELOC, D], BF16, kind="ExternalInput")
    bqk = nc.dram_tensor("bqk", [128, 2, NHP], F32, kind="ExternalInput")
    bvb = nc.dram_tensor("bvb", [128, ELOC], F32, kind="ExternalInput")
    outp = nc.dram_tensor("outp", [S, D], F32, kind="ExternalOutput")

    with tile.TileContext(nc) as tc:
        with tc.tile_pool(name="const", bufs=1) as constp, \
             tc.tile_pool(name="wpool", bufs=1) as wp, \
             tc.tile_pool(name="qkv", bufs=1) as qkvp, \
             tc.tile_pool(name="xt", bufs=1) as xtp, \
             tc.tile_pool(name="pt", bufs=6) as ptp, \
             tc.tile_pool(name="otn", bufs=8) as otnp, \
             tc.tile_pool(name="dr", bufs=2) as drp, \
             tc.tile_pool(name="osb", bufs=4) as osbp:

            # ---- constants ----
            trimask = constp.tile([128, 128], BF16, name="trimask")
            make_upper_triangular(nc, trimask[:], val=1.0, diag=True)
            ones_a = constp.tile([128, 1], BF16, name="ones_a")
            nc.gpsimd.memset(ones_a[:], 1.0)
            ones_b = constp.tile([128, 2], BF16, name="ones_b")
            nc.gpsimd.memset(ones_b[:], 0.0)
            nc.gpsimd.memset(ones_b[:, 0:1], 1.0)
            bc_lhs = constp.tile([33, 128], BF16, name="bc_lhs")
            nc.gpsimd.memset(bc_lhs[:], 0.0)
            nc.gpsimd.memset(bc_lhs[0:1, 0:64], 1.0)
            nc.gpsimd.memset(bc_lhs[32:33, 64:128], 1.0)

            bqk_sb = constp.tile([128, 2, NHP], F32, name="bqk_sb")
            nc.sync.dma_start(bqk_sb[:], bqk[:])
            bvb_sb = constp.tile([128, ELOC], F32, name="bvb_sb")
            nc.sync.dma_start(bvb_sb[:], bvb[:])

            # ---- weights + xT (order matters: V inputs first so compute
            # starts early; wq/wk next; wo needed only at first out-proj) ----
            wv_sb = []
            for kt in range(NDT):
                t = wp.tile([128, ELOC], BF16, name=f"wv{kt}")
                nc.sync.dma_start(t[:], wvT[128 * kt:128 * (kt + 1), :])
                wv_sb.append(t)
            xts = []
            for kt in range(NDT):
                t = xtp.tile([128, S], BF16, name=f"xt{kt}")
                nc.sync.dma_start(t[:], xT[128 * kt:128 * (kt + 1), :])
                xts.append(t)
            wq_sb, wk_sb = [], []
            for kt in range(NDT):
                for lst, srct, nm in ((wq_sb, wqT, "wq"), (wk_sb, wkT, "wk")):
                    t = wp.tile([128, ELOC], BF16, name=f"{nm}{kt}")
                    nc.sync.dma_start(t[:], srct[128 * kt:128 * (kt + 1), :])
                    lst.append(t)
            wo_sb = []
            for hp in range(NHP):
                t = wp.tile([128, D], BF16, name=f"wo{hp}")
                nc.sync.dma_start(t[:], woT[128 * hp:128 * (hp + 1), :])
                wo_sb.append(t)

            # ---- QKV projection ----
            QT, KT = [], []
            for hp in range(NHP):
                QT.append(qkvp.tile([128, S], BF16, name=f"qt{hp}"))
                KT.append(qkvp.tile([128, S], BF16, name=f"kt{hp}"))
            V = [qkvp.tile([128, ELOC], BF16, name=f"v{st}") for st in range(NKT)]

            def make_v(psq, st):
                ps = psq.tile([128, ELOC], F32)
                for kt in range(NDT):
                    nc.tensor.matmul(
                        ps[:],
                        lhsT=xts[kt][:, 128 * st:128 * (st + 1)],
                        rhs=wv_sb[kt][:],
                        start=(kt == 0), stop=(kt == NDT - 1))
                nc.vector.tensor_add(V[st][:], ps[:], bvb_sb[:])

            with tc.tile_pool(name="psq", bufs=6, space="PSUM") as psq:
                for st in range(4):
                    make_v(psq, st)
                for hp in range(NHP):
                    for dst, wsb, col in ((QT, wq_sb, 0), (KT, wk_sb, 1)):
                        for c in range(NQC):
                            ps = psq.tile([128, QC], F32)
                            for kt in range(NDT):
                                nc.tensor.matmul(
                                    ps[:],
                                    lhsT=wsb[kt][:, 128 * hp:128 * (hp + 1)],
                                    rhs=xts[kt][:, QC * c:QC * (c + 1)],
                                    start=(kt == 0), stop=(kt == NDT - 1))
                            nc.vector.tensor_scalar_add(
                                dst[hp][:, QC * c:QC * (c + 1)], ps[:],
                                bqk_sb[:, col, hp:hp + 1])
                for st in range(4, NKT):
                    make_v(psq, st)

            # ---- attention + output projection ----
            tri3 = trimask[:][:, None, :].broadcast_to([128, 2, 128])
            with tc.tile_pool(name="pss", bufs=2, space="PSUM") as pss, \
                 tc.tile_pool(name="pso", bufs=2, space="PSUM") as pso, \
                 tc.tile_pool(name="psd", bufs=1, space="PSUM") as psd, \
                 tc.tile_pool(name="psop", bufs=1, space="PSUM") as psop:
                for j in range(NQC):
                    otn_j = []
                    for hp in range(NHP):
                        ps_ot = pso.tile([128, QC], F32)
                        ps_d = psd.tile([128, QC], F32)
                        nkt = 4 * j + 4
                        for i in range(nkt):
                            w = 128 * (i - 4 * j) if i >= 4 * j else 0
                            last = (i == nkt - 1)
                            ps_s = pss.tile([128, 2, QC], F32)
                            for h2 in range(2):
                                nc.tensor.matmul(
                                    ps_s[:, h2, w:QC],
                                    lhsT=KT[hp][64 * h2:64 * (h2 + 1),
                                                128 * i:128 * (i + 1)],
                                    rhs=QT[hp][64 * h2:64 * (h2 + 1),
                                               QC * j + w:QC * (j + 1)],
                                    start=True, stop=True)
                            pt = ptp.tile([128, 2, QC], BF16)
                            nc.scalar.activation(pt[:, :, w:QC], ps_s[:, :, w:QC],
                                                 EXP, scale=SCALE)
                            if i >= 4 * j:
                                nc.vector.tensor_mul(
                                    pt[:, :, w:w + 128], pt[:, :, w:w + 128],
                                    tri3[:, :, :])
                            for h2 in range(2):
                                nc.tensor.matmul(
                                    ps_ot[64 * h2:64 * (h2 + 1), w:QC],
                                    lhsT=V[i][:, 64 * (2 * hp + h2):
                                              64 * (2 * hp + h2 + 1)],
                                    rhs=pt[:, h2, w:QC],
                                    start=(i == 0), stop=last,
                                    tile_position=(0, 64 * h2))
                            nc.tensor.matmul(ps_d[0:1, w:QC], lhsT=ones_a[:],
                                             rhs=pt[:, 0, w:QC],
                                             start=(i == 0), stop=last)
                            nc.tensor.matmul(ps_d[32:34, w:QC], lhsT=ones_b[:],
                                             rhs=pt[:, 1, w:QC],
                                             start=(i == 0), stop=last,
                                             tile_position=(0, 32))
                        # normalization: dr = 1/d ; broadcast via pattern matmul
                        # one sanitize pass over rows 0..32 (garbage rows are
                        # clamped finite; bc_lhs zeros them in the broadcast),
                        # then one reciprocal covering both d rows.
                        xs = drp.tile([33, QC], F32, name="xs")
                        nc.vector.tensor_scalar_max(xs[:], ps_d[0:33, :], 1e-30)
                        dr = drp.tile([33, QC], BF16)
                        with nc.allow_low_precision(reason="softmax denom bf16"):
                            nc.vector.reciprocal(dr[:], xs[:])
                        ps_bc = psop.tile([128, QC], F32, tag="psopt")
                        nc.tensor.matmul(ps_bc[:], lhsT=bc_lhs[:], rhs=dr[:],
                                         start=True, stop=True)
                        dbc = drp.tile([128, QC], BF16, name="dbc")
                        nc.vector.tensor_copy(dbc[:], ps_bc[:])
                        otn = otnp.tile([128, QC], BF16)
                        nc.vector.tensor_mul(otn[:], ps_ot[:], dbc[:])
                        otn_j.append(otn)
                    # output projection for this q chunk
                    for m in range(4):
                        s0 = QC * j + 128 * m
                        for eo in range(2):
                            ps_o = psop.tile([128, 512], F32, tag="psopt")
                            for hp in range(NHP):
                                nc.tensor.matmul(
                                    ps_o[:],
                                    lhsT=otn_j[hp][:, 128 * m:128 * (m + 1)],
                                    rhs=wo_sb[hp][:, 512 * eo:512 * (eo + 1)],
                                    start=(hp == 0), stop=(hp == NHP - 1))
                            osb = osbp.tile([128, 512], F32)
                            nc.vector.tensor_copy(osb[:], ps_o[:])
                            nc.sync.dma_start(
                                outp[s0:s0 + 128, 512 * eo:512 * (eo + 1)],
                                osb[:])
    nc.compile()
    return nc


def _get_nc():
    if "nc" not in _CACHE:
        _CACHE["nc"] = _build_nc()
    return _CACHE["nc"]


def _prep_core_inputs(x, w_qkv, b_qkv, w_out, b, hg):
    r0 = ELOC * hg
    wq = w_qkv[r0:r0 + ELOC, :]
    wk = w_qkv[D + r0:D + r0 + ELOC, :]
    wv = w_qkv[2 * D + r0:2 * D + r0 + ELOC, :]
    bq = b_qkv[r0:r0 + ELOC]
    bk = b_qkv[D + r0:D + r0 + ELOC]
    bv = b_qkv[2 * D + r0:2 * D + r0 + ELOC]

    bf = ml_dtypes.bfloat16
    bqk_arr = np.empty((128, 2, NHP), np.float32)
    bqk_arr[:, 0, :] = bq.reshape(NHP, 128).T
    bqk_arr[:, 1, :] = bk.reshape(NHP, 128).T
    return {
        "xT": np.ascontiguousarray(x[b].T).astype(bf),
        "wqT": np.ascontiguousarray(wq.T).astype(bf),
        "wkT": np.ascontiguousarray(wk.T).astype(bf),
        "wvT": np.ascontiguousarray(wv.T).astype(bf),
        "woT": np.ascontiguousarray(w_out[:, r0:r0 + ELOC].T).astype(bf),
        "bqk": bqk_arr,
        "bvb": np.tile(bv.astype(np.float32)[None, :], (128, 1)),
    }


def kernel(x, w_qkv, b_qkv, w_out, b_out, _trace=False, _trace_kwargs=None):
    x = np.asarray(x, np.float32)
    w_qkv = np.asarray(w_qkv, np.float32)
    b_qkv = np.asarray(b_qkv, np.float32)
    w_out = np.asarray(w_out, np.float32)
    b_out = np.asarray(b_out, np.float32)

    nc = _get_nc()
    in_maps = []
    for core in range(NCORES):
        b, hg = core // 2, core % 2
        in_maps.append(_prep_core_inputs(x, w_qkv, b_qkv, w_out, b, hg))

    kw = {}
    if _trace:
        kw.update(trace=True, **(_trace_kwargs or {}))
    import time
    res = None
    for attempt in range(4):
        try:
            res = bass_utils.run_bass_kernel_spmd(
                nc, in_maps, core_ids=list(range(NCORES)), **kw)
            break
        except Exception:
            if attempt == 3:
                raise
            # Transient axon/NRT device flake: reset the PJRT backend so the
            # retry starts from a clean client, like a fresh process would.
            try:
                import jax
                jax.clear_caches()
                import jax._src.xla_bridge as _xb
                _xb._clear_backends()
            except Exception:
                pass
            time.sleep(5.0 * (attempt + 1))

    out = np.empty((B, S, D), np.float32)
    for b in range(B):
        out[b] = res.results[2 * b]["outp"] + res.results[2 * b + 1]["outp"] \
            + b_out[None, :]
    if _trace:
        return out, res
    return out


# revision 11
# speedup vs baseline: 1.6054x; 1.3286x over previous
"""Causal multi-head attention (dense transformer block) on 8 Trainium2 cores.

Problem: x[4, 2048, 1024], 16 heads, head_dim 64, causal softmax attention
with QKV + output projections (torch Linear layout weights).

Sharding: 8 cores = 4 batches x 2 head-groups (8 heads each).  Each core
computes QKV projection for its 8 heads, attention, and its partial output
projection (row-parallel over w_out).  Host sums the two partials per batch
and adds b_out.

All device layouts are "transposed" so no on-device transposes are needed:
  - x is fed as xT [d, s]; Q^T/K^T are produced as [head_dim, s]
  - scores are computed as S^T [k, q]; softmax runs along partitions via
    ones-matmul column sums; O is accumulated directly as O^T [e_loc, q],
    which is exactly the lhsT the output projection needs.
Matmul inputs are bf16 (PSUM accumulation is fp32); everything else fp32.
"""

import sys

sys.path.insert(0, "/opt/trn_rl_repo")

import numpy as np
import ml_dtypes

import concourse.bass as bass
import concourse.mybir as mybir
import concourse.tile as tile
from concourse import bacc
from concourse import bass_utils
from concourse.masks import make_upper_triangular

F32 = mybir.dt.float32
BF16 = mybir.dt.bfloat16
EXP = mybir.ActivationFunctionType.Exp

B, S, D = 4, 2048, 1024
HTOT, HD = 16, 64
NCORES = 8
HLOC = HTOT // 2          # heads per core
ELOC = HLOC * HD          # 512 local embedding width
NHP = HLOC // 2           # 4 head pairs
QC = 512                  # q-chunk width
NQC = S // QC             # 4
NKT = S // 128            # 16 k tiles over sequence
NDT = D // 128            # 8 k tiles over model dim
SCALE = 1.0 / float(np.sqrt(HD))

_CACHE = {}


def _build_nc():
    nc = bacc.Bacc("TRN2", target_bir_lowering=False, debug=False)

    xT = nc.dram_tensor("xT", [D, S], BF16, kind="ExternalInput")
    wqT = nc.dram_tensor("wqT", [D, ELOC], BF16, kind="ExternalInput")
    wkT = nc.dram_tensor("wkT", [D, ELOC], BF16, kind="ExternalInput")
    wvT = nc.dram_tensor("wvT", [D, ELOC], BF16, kind="ExternalInput")
    woT = nc.dram_tensor("woT", [ELOC, D], BF16, kind="ExternalInput")
    bqk = nc.dram_tensor("bqk", [128, 2, NHP], F32, kind="ExternalInput")
    bvb = nc.dram_tensor("bvb", [128, ELOC], F32, kind="ExternalInput")
    outp = nc.dram_tensor("outp", [S, D], F32, kind="ExternalOutput")

    with tile.TileContext(nc) as tc:
        with tc.tile_pool(name="const", bufs=1) as constp, \
             tc.tile_pool(name="wpool", bufs=1) as wp, \
             tc.tile_pool(name="qkv", bufs=1) as qkvp, \
             tc.tile_pool(name="xt", bufs=1) as xtp, \
             tc.tile_pool(name="pt", bufs=6) as ptp, \
             tc.tile_pool(name="otn", bufs=8) as otnp, \
             tc.tile_pool(name="dr", bufs=2) as drp, \
             tc.tile_pool(name="osb", bufs=4) as osbp:

            # ---- constants ----
            trimask = constp.tile([128, 128], BF16, name="trimask")
            make_upper_triangular(nc, trimask[:], val=1.0, diag=True)
            ones_a = constp.tile([128, 1], BF16, name="ones_a")
            nc.gpsimd.memset(ones_a[:], 1.0)
            ones_b = constp.tile([128, 2], BF16, name="ones_b")
            nc.gpsimd.memset(ones_b[:], 0.0)
            nc.gpsimd.memset(ones_b[:, 0:1], 1.0)
            bc_lhs = constp.tile([33, 128], BF16, name="bc_lhs")
            nc.gpsimd.memset(bc_lhs[:], 0.0)
            nc.gpsimd.memset(bc_lhs[0:1, 0:64], 1.0)
            nc.gpsimd.memset(bc_lhs[32:33, 64:128], 1.0)

            bqk_sb = constp.tile([128, 2, NHP], F32, name="bqk_sb")
            nc.sync.dma_start(bqk_sb[:], bqk[:])
            bvb_sb = constp.tile([128, ELOC], F32, name="bvb_sb")
            nc.sync.dma_start(bvb_sb[:], bvb[:])

            # ---- weights + xT (order matters: V inputs first so compute
            # starts early; wq/wk next; wo needed only at first out-proj) ----
            wv_sb = []
            for kt in range(NDT):
                t = wp.tile([128, ELOC], BF16, name=f"wv{kt}")
                nc.sync.dma_start(t[:], wvT[128 * kt:128 * (kt + 1), :])
                wv_sb.append(t)
            xts = []
            for kt in range(NDT):
                t = xtp.tile([128, S], BF16, name=f"xt{kt}")
                nc.sync.dma_start(t[:], xT[128 * kt:128 * (kt + 1), :])
                xts.append(t)
            wq_sb, wk_sb = [], []
            for kt in range(NDT):
                for lst, srct, nm in ((wq_sb, wqT, "wq"), (wk_sb, wkT, "wk")):
                    t = wp.tile([128, ELOC], BF16, name=f"{nm}{kt}")
                    nc.sync.dma_start(t[:], srct[128 * kt:128 * (kt + 1), :])
                    lst.append(t)
            wo_sb = []
            for hp in range(NHP):
                t = wp.tile([128, D], BF16, name=f"wo{hp}")
                nc.sync.dma_start(t[:], woT[128 * hp:128 * (hp + 1), :])
                wo_sb.append(t)

            # ---- QKV projection ----
            QT, KT = [], []
            for hp in range(NHP):
                QT.append(qkvp.tile([128, S], BF16, name=f"qt{hp}"))
                KT.append(qkvp.tile([128, S], BF16, name=f"kt{hp}"))
            V = [qkvp.tile([128, ELOC], BF16, name=f"v{st}") for st in range(NKT)]

            def make_v(psq, st):
                ps = psq.tile([128, ELOC], F32)
                for kt in range(NDT):
                    nc.tensor.matmul(
                        ps[:],
                        lhsT=xts[kt][:, 128 * st:128 * (st + 1)],
                        rhs=wv_sb[kt][:],
                        start=(kt == 0), stop=(kt == NDT - 1))
                nc.vector.tensor_add(V[st][:], ps[:], bvb_sb[:])

            with tc.tile_pool(name="psq", bufs=6, space="PSUM") as psq:
                for st in range(4):
                    make_v(psq, st)
                for hp in range(NHP):
                    for dst, wsb, col in ((QT, wq_sb, 0), (KT, wk_sb, 1)):
                        for c in range(NQC):
                            ps = psq.tile([128, QC], F32)
                            for kt in range(NDT):
                                nc.tensor.matmul(
                                    ps[:],
                                    lhsT=wsb[kt][:, 128 * hp:128 * (hp + 1)],
                                    rhs=xts[kt][:, QC * c:QC * (c + 1)],
                                    start=(kt == 0), stop=(kt == NDT - 1))
                            nc.vector.tensor_scalar_add(
                                dst[hp][:, QC * c:QC * (c + 1)], ps[:],
                                bqk_sb[:, col, hp:hp + 1])
                for st in range(4, NKT):
                    make_v(psq, st)

            # ---- attention + output projection ----
            # Two head-pairs are processed concurrently so the PE always has
            # an independent dependency chain to run while the scalar engine
            # computes exp for the other chain (keeps the HAM clock warm).
            tri3 = trimask[:][:, None, :].broadcast_to([128, 2, 128])
            with tc.tile_pool(name="pss", bufs=2, space="PSUM") as pss, \
                 tc.tile_pool(name="psov", bufs=2, space="PSUM") as psov, \
                 tc.tile_pool(name="psd", bufs=2, space="PSUM") as psd:
                for j in range(NQC):
                    nkt = 4 * j + 4
                    otn_j = {}
                    for pair in range(2):
                        hps = (2 * pair, 2 * pair + 1)
                        ps_ot = {hp: psov.tile([128, QC], F32, tag="psov",
                                                name=f"ps_ot{hp}")
                                 for hp in hps}
                        ps_d = {hp: psd.tile([128, QC], F32, tag="psd",
                                             name=f"ps_d{hp}")
                                for hp in hps}
                        for i in range(nkt):
                            w = 128 * (i - 4 * j) if i >= 4 * j else 0
                            last = (i == nkt - 1)
                            pts = {}
                            for hp in hps:
                                ps_s = pss.tile([128, 2, QC], F32, tag="pss",
                                                name="ps_s")
                                for h2 in range(2):
                                    nc.tensor.matmul(
                                        ps_s[:, h2, w:QC],
                                        lhsT=KT[hp][64 * h2:64 * (h2 + 1),
                                                    128 * i:128 * (i + 1)],
                                        rhs=QT[hp][64 * h2:64 * (h2 + 1),
                                                   QC * j + w:QC * (j + 1)],
                                        start=True, stop=True)
                                pt = ptp.tile([128, 2, QC], BF16, tag="pt",
                                              name="pt")
                                nc.scalar.activation(pt[:, :, w:QC],
                                                     ps_s[:, :, w:QC],
                                                     EXP, scale=SCALE)
                                if i >= 4 * j:
                                    nc.vector.tensor_mul(
                                        pt[:, :, w:w + 128],
                                        pt[:, :, w:w + 128], tri3[:, :, :])
                                pts[hp] = pt
                            for hp in hps:
                                pt = pts[hp]
                                for h2 in range(2):
                                    nc.tensor.matmul(
                                        ps_ot[hp][64 * h2:64 * (h2 + 1), w:QC],
                                        lhsT=V[i][:, 64 * (2 * hp + h2):
                                                  64 * (2 * hp + h2 + 1)],
                                        rhs=pt[:, h2, w:QC],
                                        start=(i == 0), stop=last,
                                        tile_position=(0, 64 * h2))
                                nc.tensor.matmul(ps_d[hp][0:1, w:QC],
                                                 lhsT=ones_a[:],
                                                 rhs=pt[:, 0, w:QC],
                                                 start=(i == 0), stop=last)
                                nc.tensor.matmul(ps_d[hp][32:34, w:QC],
                                                 lhsT=ones_b[:],
                                                 rhs=pt[:, 1, w:QC],
                                                 start=(i == 0), stop=last,
                                                 tile_position=(0, 32))
                        for hp in hps:
                            # normalization: clamp garbage rows finite, one
                            # approx reciprocal over rows 0..32, broadcast via
                            # pattern matmul into the (reused) psd bank.
                            xs = drp.tile([33, QC], F32, name="xs")
                            nc.vector.tensor_scalar_max(
                                xs[:], ps_d[hp][0:33, :], 1e-30)
                            drf = drp.tile([33, QC], F32, name="drf")
                            nc.vector.reciprocal_approx_fast(drf[:], xs[:])
                            dr = drp.tile([33, QC], BF16)
                            with nc.allow_low_precision(reason="denom bf16"):
                                nc.vector.tensor_copy(dr[:], drf[:])
                            nc.tensor.matmul(ps_d[hp][:], lhsT=bc_lhs[:],
                                             rhs=dr[:], start=True, stop=True)
                            dbc = drp.tile([128, QC], BF16, name="dbc")
                            nc.vector.tensor_copy(dbc[:], ps_d[hp][:])
                            otn = otnp.tile([128, QC], BF16, tag="otn",
                                            name="otn")
                            nc.vector.tensor_mul(otn[:], ps_ot[hp][:], dbc[:])
                            otn_j[hp] = otn
                    # output projection for this q chunk
                    for m in range(4):
                        s0 = QC * j + 128 * m
                        for eo in range(2):
                            ps_o = psov.tile([128, 512], F32, tag="psov",
                                             name="ps_o")
                            for hp in range(NHP):
                                nc.tensor.matmul(
                                    ps_o[:],
                                    lhsT=otn_j[hp][:, 128 * m:128 * (m + 1)],
                                    rhs=wo_sb[hp][:, 512 * eo:512 * (eo + 1)],
                                    start=(hp == 0), stop=(hp == NHP - 1))
                            osb = osbp.tile([128, 512], F32)
                            nc.vector.tensor_copy(osb[:], ps_o[:])
                            nc.sync.dma_start(
                                outp[s0:s0 + 128, 512 * eo:512 * (eo + 1)],
                                osb[:])
    nc.compile()
    return nc


def _get_nc():
    if "nc" not in _CACHE:
        _CACHE["nc"] = _build_nc()
    return _CACHE["nc"]


def _prep_core_inputs(x, w_qkv, b_qkv, w_out, b, hg):
    r0 = ELOC * hg
    wq = w_qkv[r0:r0 + ELOC, :]
    wk = w_qkv[D + r0:D + r0 + ELOC, :]
    wv = w_qkv[2 * D + r0:2 * D + r0 + ELOC, :]
    bq = b_qkv[r0:r0 + ELOC]
    bk = b_qkv[D + r0:D + r0 + ELOC]
    bv = b_qkv[2 * D + r0:2 * D + r0 + ELOC]

    bf = ml_dtypes.bfloat16
    bqk_arr = np.empty((128, 2, NHP), np.float32)
    bqk_arr[:, 0, :] = bq.reshape(NHP, 128).T
    bqk_arr[:, 1, :] = bk.reshape(NHP, 128).T
    return {
        "xT": np.ascontiguousarray(x[b].T).astype(bf),
        "wqT": np.ascontiguousarray(wq.T).astype(bf),
        "wkT": np.ascontiguousarray(wk.T).astype(bf),
        "wvT": np.ascontiguousarray(wv.T).astype(bf),
        "woT": np.ascontiguousarray(w_out[:, r0:r0 + ELOC].T).astype(bf),
        "bqk": bqk_arr,
        "bvb": np.tile(bv.astype(np.float32)[None, :], (128, 1)),
    }


def kernel(x, w_qkv, b_qkv, w_out, b_out, _trace=False, _trace_kwargs=None):
    x = np.asarray(x, np.float32)
    w_qkv = np.asarray(w_qkv, np.float32)
    b_qkv = np.asarray(b_qkv, np.float32)
    w_out = np.asarray(w_out, np.float32)
    b_out = np.asarray(b_out, np.float32)

    nc = _get_nc()
    in_maps = []
    for core in range(NCORES):
        b, hg = core // 2, core % 2
        in_maps.append(_prep_core_inputs(x, w_qkv, b_qkv, w_out, b, hg))

    kw = {}
    if _trace:
        kw.update(trace=True, **(_trace_kwargs or {}))
    import time
    res = None
    for attempt in range(4):
        try:
            res = bass_utils.run_bass_kernel_spmd(
                nc, in_maps, core_ids=list(range(NCORES)), **kw)
            break
        except Exception:
            if attempt == 3:
                raise
            # Transient axon/NRT device flake: reset the PJRT backend so the
            # retry starts from a clean client, like a fresh process would.
            try:
                import jax
                jax.clear_caches()
                import jax._src.xla_bridge as _xb
                _xb._clear_backends()
            except Exception:
                pass
            time.sleep(5.0 * (attempt + 1))

    out = np.empty((B, S, D), np.float32)
    for b in range(B):
        out[b] = res.results[2 * b]["outp"] + res.results[2 * b + 1]["outp"] \
            + b_out[None, :]
    if _trace:
        return out, res
    return out
